# revision 1
# baseline (speedup 1.0000x reference)
"""Self-contained Trainium2 kernel for nn_DecoderOnlyTransformer_10239202034008.

Sharding: 8 cores = 4 pairs; pair p owns batch element p. Within a pair,
tokens are zigzag-chunk sharded (balanced causal work, SPMD-uniform program).
Residual stream lives feature-major (xT [D, SH]) in SBUF fp32; matmuls bf16.
"""

import math
from contextlib import ExitStack
from types import SimpleNamespace

import numpy as np
import ml_dtypes

import concourse.bass as bass
import concourse.mybir as mybir
import concourse.tile as tile
from concourse import bacc
from concourse.bass_utils import run_bass_kernel_spmd

P = 128
F32 = mybir.dt.float32
BF16 = mybir.dt.bfloat16


def make_cfg(B=4, S=2048, D=512, H=8, L=6, V=8000, FFM=4, vsl=500):
    c = SimpleNamespace()
    c.B, c.S, c.D, c.H, c.L, c.V, c.FFM = B, S, D, H, L, V, FFM
    c.HD = D // H
    c.FF = FFM * D
    c.NCH = S // P          # chunks per sequence
    c.SH = S // 2           # tokens per core
    c.NSLOT = c.NCH // 2    # q-slots per core
    c.FB = D // P           # feature blocks
    c.FFB = c.FF // P       # ff blocks
    c.TB = c.SH // P        # local token blocks
    c.NCORES = 2 * B
    c.TH = max(1, c.SH // 512)   # token superblocks of <=512
    c.THW = c.SH // c.TH         # superblock width
    c.VSL = vsl                  # vocab slice width
    c.NVS = (V + vsl - 1) // vsl
    assert V % vsl == 0
    c.eps = 1e-5
    c.inv_scale = 1.0 / math.sqrt(D)
    # zigzag chunk assignment (rank0/rank1 of each pair), slots sorted by
    # descending causal extent; E = uniform per-slot k-extent (max over ranks).
    k = c.NCH // 2
    A = [x for x in range(0, k) if x % 2 == 0] + [x for x in range(k, c.NCH) if x % 2 == 1]
    Bset = [x for x in range(0, k) if x % 2 == 1] + [x for x in range(k, c.NCH) if x % 2 == 0]
    A = sorted(A, key=lambda x: -x)
    Bset = sorted(Bset, key=lambda x: -x)
    c.slot_chunks = {0: A, 1: Bset}
    c.E = [max(a, b) + 1 for a, b in zip(A, Bset)]
    for r in (0, 1):
        for s in range(c.NSLOT):
            assert 0 <= c.E[s] - (c.slot_chunks[r][s] + 1) <= 1
    # physical chunk -> AG-buffer token offset (identical on both ranks)
    c.chunk_off = {}
    for r in (0, 1):
        for s, ch in enumerate(c.slot_chunks[r]):
            c.chunk_off[ch] = r * c.SH + s * P
    # active slot count at k-iteration t
    c.m_of_t = [sum(1 for e in c.E if e > t) for t in range(c.NCH)]
    return c


def _halves(n, w=512):
    out = []
    o = 0
    while o < n:
        out.append((o, min(w, n - o)))
        o += w
    return out


def build_program(c):
    nc = bacc.Bacc("TRN2", target_bir_lowering=False, debug=False,
                   num_devices=c.NCORES)

    x0T = nc.dram_tensor("x0T", [c.D, c.SH], F32, kind="ExternalInput").ap()
    masks = nc.dram_tensor("masks", [c.NCH, P, P], BF16, kind="ExternalInput").ap()
    wq = nc.dram_tensor("wq", [c.L, c.D, c.D], BF16, kind="ExternalInput").ap()
    wk = nc.dram_tensor("wk", [c.L, c.D, c.D], BF16, kind="ExternalInput").ap()
    wv = nc.dram_tensor("wv", [c.L, c.D, c.D], BF16, kind="ExternalInput").ap()
    wo = nc.dram_tensor("wo", [c.L, c.D, c.D], BF16, kind="ExternalInput").ap()
    w1 = nc.dram_tensor("w1", [c.L, c.D, c.FF], BF16, kind="ExternalInput").ap()
    w2 = nc.dram_tensor("w2", [c.L, c.FF, c.D], BF16, kind="ExternalInput").ap()
    b1 = nc.dram_tensor("b1", [c.L, c.FF], F32, kind="ExternalInput").ap()
    bo = nc.dram_tensor("bo", [c.L, c.D], F32, kind="ExternalInput").ap()
    b2 = nc.dram_tensor("b2", [c.L, c.D], F32, kind="ExternalInput").ap()
    wout = nc.dram_tensor("wout", [c.D, c.V], BF16, kind="ExternalInput").ap()
    logits = nc.dram_tensor("logits", [c.SH, c.V], F32, kind="ExternalOutput").ap()

    groups = [[2 * i, 2 * i + 1] for i in range(c.NCORES // 2)]

    with tile.TileContext(nc) as tc:
        _body(tc, c, x0T, masks, wq, wk, wv, wo, w1, w2, b1, bo, b2,
              wout, logits, groups)
    nc.compile()
    return nc


def _body(tc, c, x0T, masks, wq, wk, wv, wo, w1, w2, b1, bo, b2,
          wout, logits, groups):
    nc = tc.nc
    SH, FB, FFB, TB, NCH, H, HD = c.SH, c.FB, c.FFB, c.TB, c.NCH, c.H, c.HD
    AF = mybir.ActivationFunctionType
    OP = mybir.AluOpType

    ctx = ExitStack()
    pers = ctx.enter_context(tc.tile_pool(name="pers", bufs=1))
    dbl = ctx.enter_context(tc.tile_pool(name="dbl", bufs=2))
    ppool = ctx.enter_context(tc.tile_pool(name="ps", bufs=4, space="PSUM"))
    dram = ctx.enter_context(tc.tile_pool(name="dram", bufs=2, space="DRAM"))

    def psum(part, free):
        t = ppool.tile([P, 1024], F32, tag="ps", name="ps")
        return t[:part, :free]

    # constants
    ones_red_f = pers.tile([P, 1], F32, tag="ones_red_f")
    nc.gpsimd.memset(ones_red_f[:], 1.0)
    ones_red_b = pers.tile([P, 1], BF16, tag="ones_red_b")
    nc.gpsimd.memset(ones_red_b[:], 1.0)
    ones_col = pers.tile([1, P], F32, tag="ones_col")
    nc.gpsimd.memset(ones_col[:], 1.0)
    masks_sb = pers.tile([P, NCH, P], BF16, tag="masks")
    nc.sync.dma_start(masks_sb[:], masks.rearrange("t p q -> p t q"))
    # scratch stat rows (partition 0): shared by LN and softmax recip
    rows = pers.tile([1, 3, SH], F32, tag="rows")

    xT = pers.tile([P, FB, SH], F32, tag="xT")
    nc.sync.dma_start(xT[:], x0T.rearrange("(fb p) t -> p fb t", p=P))

    def ln_pass(src):
        """LayerNorm (g/b folded into weights host-side): src fp32 -> hT bf16."""
        hT = pers.tile([P, FB, SH], BF16, tag="hT", name="hT")
        st0 = psum(1, SH)  # sum(x)
        st1 = psum(1, SH)  # sum(x^2)
        xsq = pers.tile([P, FB, SH], BF16, tag="xsq", name="xsq")
        nc.scalar.activation(xsq[:], src[:], AF.Square)
        for (o, w) in _halves(SH):
            for fb in range(FB):
                nc.tensor.matmul(st0[0:1, o:o + w], ones_red_f, src[:, fb, o:o + w],
                                 start=(fb == 0), stop=(fb == FB - 1))
            for fb in range(FB):
                nc.tensor.matmul(st1[0:1, o:o + w], ones_red_b, xsq[:, fb, o:o + w],
                                 start=(fb == 0), stop=(fb == FB - 1))
        nm = rows[0:1, 0, :]    # -mean
        r1 = rows[0:1, 1, :]    # ex2 -> var -> rstd
        r2 = rows[0:1, 2, :]    # mean^2 -> log(var) -> -mean*rstd
        nc.vector.tensor_scalar_mul(nm, st0[0:1, :SH], -1.0 / c.D)
        nc.vector.tensor_scalar(r1, st1[0:1, :SH], 1.0 / c.D, float(c.eps),
                                OP.mult, OP.add)
        nc.vector.tensor_mul(r2, nm, nm)
        nc.vector.tensor_sub(r1, r1, r2)          # var + eps
        nc.scalar.activation(r2, r1, AF.Ln)
        nc.scalar.activation(r1, r2, AF.Exp, scale=-0.5)   # rstd
        nc.vector.tensor_mul(r2, nm, r1)          # -mean*rstd
        for (o, w) in _halves(SH):
            rb = psum(P, w)
            nc.tensor.matmul(rb[:, :w], ones_col, r1[:, o:o + w],
                             start=True, stop=True)
            mb = psum(P, w)
            nc.tensor.matmul(mb[:, :w], ones_col, r2[:, o:o + w],
                             start=True, stop=True)
            for fb in range(FB):
                nc.vector.tensor_mul(hT[:, fb, o:o + w], src[:, fb, o:o + w], rb[:, :w])
                nc.vector.tensor_add(hT[:, fb, o:o + w], hT[:, fb, o:o + w], mb[:, :w])
        return hT

    def proj_featmajor(dst, w_sb, hT, cast_eng):
        """dst[P, FB, SH] (feat-major) = w.T @ h via lhsT=weight slices."""
        for dblk in range(FB):
            ps = psum(P, SH)
            for (o, w) in _halves(SH):
                for ks in range(FB):
                    nc.tensor.matmul(ps[:, o:o + w],
                                     w_sb[:, ks, dblk * P:(dblk + 1) * P],
                                     hT[:, ks, o:o + w],
                                     start=(ks == 0), stop=(ks == FB - 1))
            if cast_eng == "v":
                nc.vector.tensor_copy(dst[:, dblk, :], ps[:, :SH])
            else:
                nc.scalar.copy(dst[:, dblk, :], ps[:, :SH])

    for l in range(c.L):
        hT = ln_pass(xT)

        # ---- QKV projections (weights streamed through one double-buffered tag)
        wq_sb = dbl.tile([P, FB, c.D], BF16, tag="wmat", name="wq_sb")
        nc.sync.dma_start(wq_sb[:], wq[l].rearrange("(ks p) n -> p ks n", p=P))
        qT = pers.tile([P, FB, SH], BF16, tag="qT", name="qT")
        proj_featmajor(qT, wq_sb, hT, "v")

        wk_sb = dbl.tile([P, FB, c.D], BF16, tag="wmat", name="wk_sb")
        nc.sync.dma_start(wk_sb[:], wk[l].rearrange("(ks p) n -> p ks n", p=P))
        kT_sb = pers.tile([P, FB, SH], BF16, tag="kT_sb", name="kT_sb")
        proj_featmajor(kT_sb, wk_sb, hT, "s")

        wv_sb = dbl.tile([P, FB, c.D], BF16, tag="wmat", name="wv_sb")
        nc.sync.dma_start(wv_sb[:], wv[l].rearrange("(ks p) n -> p ks n", p=P))
        v_sb = pers.tile([P, TB, H, HD + 1], BF16, tag="v_sb", name="v_sb")
        nc.gpsimd.memset(v_sb[:, :, :, HD:HD + 1], 1.0)
        for tb in range(TB):
            ps = psum(P, c.D)
            for ks in range(FB):
                nc.tensor.matmul(ps[:, :c.D], hT[:, ks, tb * P:(tb + 1) * P],
                                 wv_sb[:, ks, :], start=(ks == 0), stop=(ks == FB - 1))
            nc.vector.tensor_copy(
                v_sb[:, tb, :, 0:HD],
                ps[:, :c.D].rearrange("p (h d) -> p h d", h=H))

        # ---- AllGather kT + v across the pair ----
        KW = FB * SH
        VW = TB * H * (HD + 1)
        kv_d = dram.tile([P, KW + VW], BF16, tag="kv_d")
        kvg_d = dram.tile([2, P, KW + VW], BF16, tag="kvg_d")
        nc.sync.dma_start(kv_d[:, :KW].rearrange("p (fb t) -> p fb t", fb=FB),
                          kT_sb[:])
        nc.sync.dma_start(
            kv_d[:, KW:].rearrange("p (tb h d) -> p tb h d", tb=TB, h=H),
            v_sb[:])
        if __import__('os').environ.get('NO_COLLECTIVE'):
            nc.sync.dma_start(kvg_d[0], kv_d[:])
            nc.sync.dma_start(kvg_d[1], kv_d[:])
        else:
            nc.gpsimd.collective_compute(
                "AllGather", OP.bypass, replica_groups=groups,
                ins=[kv_d[:].opt()], outs=[kvg_d[:].opt()])
        kTf = pers.tile([P, FB, 2 * SH], BF16, tag="kTf", name="kTf")
        vpad = pers.tile([P, NCH, H, HD + 1], BF16, tag="vpad", name="vpad")
        for r in range(2):
            nc.sync.dma_start(
                kTf[:, :, r * SH:(r + 1) * SH],
                kvg_d[r, :, :KW].rearrange("p (fb t) -> p fb t", fb=FB))
            nc.sync.dma_start(
                vpad[:, r * TB:(r + 1) * TB],
                kvg_d[r, :, KW:].rearrange("p (tb h d) -> p tb h d", tb=TB, h=H))

        # ---- attention (scores transposed: [k, q]) ----
        oT = pers.tile([P, FB, SH], BF16, tag="oT", name="oT")
        for h in range(H):
            fbh, rh = h // 2, (h % 2) * HD
            o_ps = psum(HD + 1, SH)
            stop_t = {o: max(t for t in range(NCH) if c.m_of_t[t] * P > o)
                      for (o, w) in _halves(SH)}
            for t in range(NCH):
                mq = c.m_of_t[t] * P
                ko = c.chunk_off[t]
                sc = psum(P, mq)
                for (o, w) in _halves(mq):
                    nc.tensor.matmul(sc[:, o:o + w],
                                     kTf[rh:rh + HD, fbh, ko:ko + P],
                                     qT[rh:rh + HD, fbh, o:o + w],
                                     start=True, stop=True)
                pT = dbl.tile([P, SH], BF16, tag="pT", name="pT")
                nc.scalar.activation(pT[:, :mq], sc[:, :mq], AF.Exp,
                                     scale=float(c.inv_scale))
                sl = c.m_of_t[t] - 1
                nc.vector.tensor_mul(pT[:, sl * P:(sl + 1) * P],
                                     pT[:, sl * P:(sl + 1) * P],
                                     masks_sb[:, t, :])
                for (o, w) in _halves(mq):
                    nc.tensor.matmul(o_ps[:, o:o + w],
                                     vpad[:, ko // P, h, :],
                                     pT[:, o:o + w],
                                     start=(t == 0), stop=(t == stop_t[o]))
            sums = rows[0:1, 0, :]
            lsum = rows[0:1, 1, :]
            recip = rows[0:1, 2, :]
            nc.scalar.copy(sums, o_ps[HD:HD + 1, :SH])
            nc.scalar.activation(lsum, sums, AF.Ln)
            nc.scalar.activation(recip, lsum, AF.Exp, scale=-1.0)
            for (o, w) in _halves(SH):
                rb = psum(HD, w)
                nc.tensor.matmul(rb[:, :w], ones_col[0:1, :HD],
                                 recip[:, o:o + w], start=True, stop=True)
                rb_sb = dbl.tile([HD, 512], F32, tag="rb_sb", name="rb_sb")
                nc.scalar.copy(rb_sb[:, :w], rb[:, :w])
                nc.vector.tensor_mul(oT[rh:rh + HD, fbh, o:o + w],
                                     o_ps[:HD, o:o + w], rb_sb[:, :w])

        # ---- wo projection + residual ----
        wo_sb = dbl.tile([P, FB, c.D], BF16, tag="wmat", name="wo_sb")
        nc.sync.dma_start(wo_sb[:], wo[l].rearrange("(ks p) n -> p ks n", p=P))
        bo_sb = dbl.tile([P, FB], F32, tag="bo", name="bo_sb")
        nc.sync.dma_start(bo_sb[:], bo[l].rearrange("(fb p) -> p fb", p=P))
        for dblk in range(FB):
            ps = psum(P, SH)
            for (o, w) in _halves(SH):
                for ks in range(FB):
                    nc.tensor.matmul(ps[:, o:o + w],
                                     wo_sb[:, ks, dblk * P:(dblk + 1) * P],
                                     oT[:, ks, o:o + w],
                                     start=(ks == 0), stop=(ks == FB - 1))
            tmp = dbl.tile([P, SH], F32, tag="res_tmp", name="res_tmp")
            nc.scalar.activation(tmp[:], ps[:, :SH], AF.Copy)
            nc.vector.tensor_scalar_add(tmp[:], tmp[:], bo_sb[:, dblk:dblk + 1])
            nc.vector.tensor_add(xT[:, dblk, :], xT[:, dblk, :], tmp[:])

        # ---- FFN ----
        w1_sb = pers.tile([P, FB, c.FF], BF16, tag="w1", name="w1_sb")
        nc.sync.dma_start(w1_sb[:], w1[l].rearrange("(ks p) n -> p ks n", p=P))
        b1_sb = dbl.tile([P, FFB], F32, tag="b1", name="b1_sb")
        nc.sync.dma_start(b1_sb[:], b1[l].rearrange("(fb p) -> p fb", p=P))
        b2_sb = dbl.tile([P, FB], F32, tag="b2", name="b2_sb")
        nc.sync.dma_start(b2_sb[:], b2[l].rearrange("(fb p) -> p fb", p=P))
        h2T = ln_pass(xT)
        W2C = 4  # ffb chunk for streaming w2
        for (o, w) in _halves(SH):
            aT = pers.tile([P, FFB, c.THW], BF16, tag="aT", name="aT")
            for fb in range(FFB):
                ps = psum(P, w)
                for ks in range(FB):
                    nc.tensor.matmul(ps[:, :w], w1_sb[:, ks, fb * P:(fb + 1) * P],
                                     h2T[:, ks, o:o + w],
                                     start=(ks == 0), stop=(ks == FB - 1))
                nc.vector.tensor_scalar(aT[:, fb, :w], ps[:, :w],
                                        b1_sb[:, fb:fb + 1], 0.0,
                                        OP.add, OP.max)
            y_ps = [psum(P, w) for _ in range(FB)]
            for fc in range(0, FFB, W2C):
                w2_sb = dbl.tile([P, W2C, c.D], BF16, tag="w2c", name="w2_sb")
                nc.sync.dma_start(
                    w2_sb[:],
                    w2[l, fc * P:(fc + W2C) * P].rearrange(
                        "(fb p) n -> p fb n", p=P))
                for fb in range(W2C):
                    for dblk in range(FB):
                        nc.tensor.matmul(
                            y_ps[dblk][:, :w],
                            w2_sb[:, fb, dblk * P:(dblk + 1) * P],
                            aT[:, fc + fb, :w],
                            start=(fc + fb == 0), stop=(fc + fb == FFB - 1))
            for dblk in range(FB):
                tmp = dbl.tile([P, SH], F32, tag="res_tmp", name="res_tmp")
                nc.scalar.activation(tmp[:, :w], y_ps[dblk][:, :w], AF.Copy)
                nc.vector.tensor_scalar_add(tmp[:, :w], tmp[:, :w],
                                            b2_sb[:, dblk:dblk + 1])
                nc.vector.tensor_add(xT[:, dblk, o:o + w], xT[:, dblk, o:o + w],
                                     tmp[:, :w])

    # ---- final LN + vocab projection ----
    hT = ln_pass(xT)
    for vs in range(c.NVS):
        wo_t = dbl.tile([P, FB, c.VSL], BF16, tag="wout", name="wo_t")
        nc.sync.dma_start(wo_t[:],
                          wout[:, vs * c.VSL:(vs + 1) * c.VSL]
                          .rearrange("(ks p) n -> p ks n", p=P))
        for tb in range(TB):
            ps = psum(P, c.VSL)
            for ks in range(FB):
                nc.tensor.matmul(ps[:, :c.VSL], hT[:, ks, tb * P:(tb + 1) * P],
                                 wo_t[:, ks, :], start=(ks == 0), stop=(ks == FB - 1))
            lg_sb = dbl.tile([P, c.VSL], F32, tag="lg_sb", name="lg_sb")
            if tb % 2 == 0:
                nc.vector.tensor_copy(lg_sb[:], ps[:, :c.VSL])
            else:
                nc.scalar.copy(lg_sb[:], ps[:, :c.VSL])
            nc.sync.dma_start(
                logits[tb * P:(tb + 1) * P, vs * c.VSL:(vs + 1) * c.VSL], lg_sb[:])

    ctx.close()


# ---------------- host side ----------------

def _pos_encoding(S, D):
    pos = np.arange(S, dtype=np.float32)[:, None]
    div = np.exp(np.arange(0, D, 2, dtype=np.float32) * (-np.log(10000.0) / D))
    pe = np.zeros((S, D), np.float32)
    pe[:, 0::2] = np.sin(pos * div)
    pe[:, 1::2] = np.cos(pos * div)
    return pe


def _bf(x):
    return np.asarray(x, np.float32).astype(ml_dtypes.bfloat16)


def make_inputs(c, tokens, embed, wq, wk, wv, wo, bo, w1, b1, w2, b2,
                ln_g, ln_b, lnf_g, lnf_b, w_out, b_out):
    """Host preprocessing -> per-core input maps + host-side residual row."""
    tokens = np.asarray(tokens)
    x0 = np.asarray(embed, np.float32)[tokens] + _pos_encoding(c.S, c.D)[None]
    ln_g = np.asarray(ln_g, np.float32)
    ln_b = np.asarray(ln_b, np.float32)
    wq_f = _bf(np.asarray(wq, np.float32) * ln_g[:, :, None])
    wk_f = _bf(np.asarray(wk, np.float32) * ln_g[:, :, None])
    wv_f = _bf(np.asarray(wv, np.float32) * ln_g[:, :, None])
    w1_f = _bf(np.asarray(w1, np.float32) * ln_g[:, :, None])
    wo_f = _bf(wo)
    w2_f = _bf(w2)
    wout_f = _bf(np.asarray(w_out, np.float32) * np.asarray(lnf_g, np.float32)[:, None])
    assert not np.any(ln_b), "nonzero ln_b not supported by this kernel"
    b1_f = np.asarray(b1, np.float32)
    bo_f = np.asarray(bo, np.float32)
    b2_f = np.asarray(b2, np.float32)
    out_row = (np.asarray(lnf_b, np.float32) @ np.asarray(w_out, np.float32)
               + np.asarray(b_out, np.float32))
    assert not np.any(lnf_b), "nonzero lnf_b not supported by this kernel"

    in_maps = []
    for core in range(c.NCORES):
        r, bp = core % 2, core // 2
        chunks = c.slot_chunks[r]
        tok_rows = np.concatenate(
            [np.arange(ch * P, (ch + 1) * P) for ch in chunks])
        x0T = np.ascontiguousarray(x0[bp, tok_rows, :].T.astype(np.float32))
        # masks[t]: [k within block, q within block] for the last-active slot
        mk = np.zeros((c.NCH, P, P), np.float32)
        for t in range(c.NCH):
            s = c.m_of_t[t] - 1
            ch = chunks[s]
            e = ch + 1
            if t < e - 1:
                mk[t] = 1.0
            elif t == e - 1:
                kk = np.arange(P)[:, None]
                qq = np.arange(P)[None, :]
                mk[t] = (kk <= qq).astype(np.float32)
            # t >= e: zeros
        in_maps.append({
            "x0T": x0T,
            "masks": mk.astype(ml_dtypes.bfloat16),
            "wq": wq_f, "wk": wk_f, "wv": wv_f, "wo": wo_f,
            "w1": w1_f, "w2": w2_f,
            "b1": b1_f, "bo": bo_f, "b2": b2_f,
            "wout": wout_f,
        })
    return in_maps, out_row


def gather_output(c, results, out_row):
    out = np.zeros((c.B, c.S, c.V), np.float32)
    for core in range(c.NCORES):
        r, bp = core % 2, core // 2
        chunks = c.slot_chunks[r]
        lg = np.asarray(results[core]["logits"], np.float32)
        for s, ch in enumerate(chunks):
            out[bp, ch * P:(ch + 1) * P, :] = lg[s * P:(s + 1) * P, :]
    if np.any(out_row):
        out += out_row[None, None, :]
    return out


_CACHE = {}


def run(inputs, trace=False):
    c = make_cfg()
    in_maps, out_row = make_inputs(c, **inputs)
    if "nc" not in _CACHE:
        _CACHE["nc"] = build_program(c)
    res = run_bass_kernel_spmd(_CACHE["nc"], in_maps,
                               core_ids=list(range(c.NCORES)), trace=trace)
    return gather_output(c, res.results, out_row), res


def kernel(**inputs):
    return run(inputs)[0]

